# revision 44
# baseline (speedup 1.0000x reference)
"""GPT-Neo self-attention on 8 NeuronCores (Trainium2, Bass/Tile) — v10.

Sharding: core i handles batch i//4 and head-group i%4 (3 of 12 heads).
Each core computes a partial out-projection [S, D] (bf16); host sums the
4 partials per batch in f32.

v5 vs v4:
- Per-chunk SBUF tiles for q/k/v/onorm/ot so the tile-granularity
  dependency tracker cannot create false cross-chunk waits (PE matmuls
  were stalling on unrelated copies into shared tiles).
- Triangle PVs + normalize for head h are DEFERRED one head: their
  exp -> GpSimd-mask -> PV chain gets a whole head of slack, so the
  in-order PE stream never waits on the Pool/DVE queues.
- Ascending chunks; proj(c+2) psum-groups and out-projections run as
  PE fillers inside att windows (they need no ACT work, balancing the
  exp-bound attention phases).
"""

import numpy as np
import ml_dtypes
from collections import deque
from contextlib import ExitStack

import concourse.bass as bass
from concourse import bacc
import concourse.mybir as mybir
import concourse.tile as tile
from concourse.bass_utils import run_bass_kernel_spmd

B, S, D, H = 2, 2048, 768, 12
HD = 64
HPC = 3
NCORES = 8
NEG = -1.0e30
F32 = mybir.dt.float32
BF16 = mybir.dt.bfloat16
EXP = mybir.ActivationFunctionType.Exp
COPY = mybir.ActivationFunctionType.Copy

KT = D // 128
SQT = S // 128
CH = S // 512
LAG = 6          # main-PV pair-units behind scores


def build_nc(use_pbias=False):
    nc = bacc.Bacc(None, target_bir_lowering=False)

    xT = [nc.declare_dram_parameter(f"xT{c}", [128, KT, 512], BF16,
                                    isOutput=False) for c in range(CH)]
    wqk = nc.declare_dram_parameter("wqk", [128, KT, 384], BF16, isOutput=False)
    wv = nc.declare_dram_parameter("wv", [128, KT, 192], BF16, isOutput=False)
    wop = nc.declare_dram_parameter("wop", [128, D], BF16, isOutput=False)
    wos = nc.declare_dram_parameter("wos", [65, D], BF16, isOutput=False)
    trid = nc.declare_dram_parameter("trid", [128, 2, 128], BF16, isOutput=False)
    if use_pbias:
        pbias = nc.declare_dram_parameter("pbias", [128, SQT], F32, isOutput=False)
    y = nc.declare_dram_parameter("y", [S, D], BF16, isOutput=True)

    with tile.TileContext(nc) as tc:
        with ExitStack() as ctx:
            persist = ctx.enter_context(tc.tile_pool(name="persist", bufs=1))
            ptp = ctx.enter_context(tc.tile_pool(name="ptp", bufs=14))
            recp = ctx.enter_context(tc.tile_pool(name="recp", bufs=3))
            posp = ctx.enter_context(tc.tile_pool(name="posp", bufs=2))
            bcp = ctx.enter_context(tc.tile_pool(name="bcp", bufs=2))
            big = ctx.enter_context(tc.tile_pool(name="big", bufs=3, space="PSUM"))
            pop = ctx.enter_context(tc.tile_pool(name="pop", bufs=2, space="PSUM"))

            xc = [persist.tile([128, KT, 512], BF16, tag=f"xc{c}", name=f"xc{c}")
                  for c in range(CH)]
            wqk_sb = persist.tile([128, KT, 384], BF16, tag="wqk", name="wqk")
            wv_sb = persist.tile([128, KT, 192], BF16, tag="wv", name="wv")
            wop_sb = persist.tile([128, D], BF16, tag="wop", name="wop")
            wos_sb = persist.tile([65, D], BF16, tag="wos", name="wos")
            tri_sb = persist.tile([128, 2, 128], BF16, tag="tri", name="tri")
            ones65 = persist.tile([1, 65], BF16, tag="ones65", name="ones65")
            if use_pbias:
                pb_sb = persist.tile([128, SQT], F32, tag="pb", name="pb")
            # per-chunk activation tiles (avoid false tile-level deps)
            q01c = [persist.tile([128, 512], BF16, tag=f"q01_{c}", name=f"q01_{c}")
                    for c in range(CH)]
            k01c = [persist.tile([128, 512], BF16, tag=f"k01_{c}", name=f"k01_{c}")
                    for c in range(CH)]
            q2c = [persist.tile([64, 512], BF16, tag=f"q2_{c}", name=f"q2_{c}")
                   for c in range(CH)]
            k2c = [persist.tile([64, 512], BF16, tag=f"k2_{c}", name=f"k2_{c}")
                   for c in range(CH)]
            vc = [persist.tile([128, HPC, 4, 65], BF16, tag=f"v{c}", name=f"v{c}")
                  for c in range(CH)]
            onp = [persist.tile([128, 512], BF16, tag=f"onp{c}", name=f"onp{c}")
                   for c in range(CH)]
            ons = [persist.tile([65, 512], BF16, tag=f"ons{c}", name=f"ons{c}")
                   for c in range(CH)]
            otc = [persist.tile([128, 4, D], BF16, tag=f"ot{c}", name=f"ot{c}")
                   for c in range(CH)]

            # ---- input DMAs on both DGE queues ----
            # only proj(0)'s inputs before the first matmuls: the first
            # compute's DMA-counter wait then covers just these transfers
            nc.sync.dma_start(out=wqk_sb[:], in_=wqk[:, :, :])
            nc.scalar.dma_start(out=wv_sb[:], in_=wv[:, :, :])
            nc.sync.dma_start(out=xc[0][:, 0:3, :], in_=xT[0][:, 0:3, :])
            nc.scalar.dma_start(out=xc[0][:, 3:6, :], in_=xT[0][:, 3:6, :])
            for c in range(CH):
                nc.vector.memset(vc[c][:], 1.0)
            nc.vector.memset(ones65[:], 1.0)

            def proj_qk_group(c, off, on_act=True):
                ps = big.tile([128, 1024], F32, tag="big", name=f"p{off}_{c}")
                for k in range(KT):
                    nc.tensor.matmul(
                        out=ps[:, 0:512],
                        lhsT=wqk_sb[:, k, off:off + 128],
                        rhs=xc[c][:, k, :],
                        start=(k == 0), stop=(k == KT - 1))
                cp = (nc.scalar.copy if on_act else
                      (lambda out, in_: nc.vector.tensor_copy(out=out, in_=in_)))
                if off == 0:
                    cp(out=q01c[c][:], in_=ps[:, 0:512])
                elif off == 128:
                    cp(out=k01c[c][:], in_=ps[:, 0:512])
                else:
                    cp(out=q2c[c][:], in_=ps[0:64, 0:512])
                    nc.vector.tensor_copy(out=k2c[c][:], in_=ps[64:128, 0:512])

            def proj_v_group(c, jj):
                pv = big.tile([128, 1024], F32, tag="big", name=f"pv{c}_{jj}")
                for k in range(KT):
                    nc.tensor.matmul(
                        out=pv[:, 0:192],
                        lhsT=xc[c][:, k, 128 * jj:128 * (jj + 1)],
                        rhs=wv_sb[:, k, :],
                        start=(k == 0), stop=(k == KT - 1))
                for h in range(HPC):
                    nc.vector.tensor_copy(
                        out=vc[c][:, h, jj, 0:64],
                        in_=pv[:, 64 * h:64 * (h + 1)])

            def proj_groups(c, on_act=True):
                gs = [lambda off=off: proj_qk_group(c, off, on_act)
                      for off in (0, 128, 256)]
                gs += [lambda jj=jj: proj_v_group(c, jj) for jj in range(4)]
                return gs

            def proj(c):
                for g in proj_groups(c):
                    g()

            def kq(h, j):
                """(k-block lhsT, q-chunk-view fn) for head h, key tile j."""
                cj, jj = j // 4, j % 4
                if h == 2:
                    return k2c[cj][:, 128 * jj:128 * (jj + 1)]
                lo = 64 * h
                return k01c[cj][lo:lo + 64, 128 * jj:128 * (jj + 1)]

            def qv(h, c, lo, hi):
                if h == 2:
                    return q2c[c][:, lo:hi]
                p0 = 64 * h
                return q01c[c][p0:p0 + 64, lo:hi]

            def v_ap(h, j):
                return vc[j // 4][:, h, j % 4, :]

            def exp_emit(pt, Sg, sections):
                if use_pbias:
                    for lo, hi, j in sections:
                        nc.scalar.activation(out=pt[:, lo:hi], in_=Sg[:, lo:hi],
                                             func=EXP, bias=pb_sb[:, j:j + 1])
                else:
                    lo, hi = sections[0][0], sections[-1][1]
                    nc.scalar.activation(out=pt[:, lo:hi], in_=Sg[:, lo:hi],
                                         func=EXP)

            def tri_mask(pt, stride):
                """one Pool multiply masking cols {0:128, stride:stride+128}"""
                v = pt[:].rearrange("p (a b) -> p a b", b=stride)[:, 0:2, 0:128]
                nc.gpsimd.tensor_mul(out=v, in0=v, in1=tri_sb[:])

            def outproj(t, tail=False):
                c_, tt = t // 4, t % 4
                Sg = big.tile([128, 1024], F32, tag="big", name=f"op{t}")
                ts_ = slice(128 * tt, 128 * (tt + 1))
                for lo in (0, 512):
                    hs = slice(384 * (lo // 512), 384 * (lo // 512) + 384)
                    nc.tensor.matmul(out=Sg[:, lo:lo + 384],
                                     lhsT=onp[c_][:, ts_], rhs=wop_sb[:, hs],
                                     start=True, stop=False)
                    nc.tensor.matmul(out=Sg[:, lo:lo + 384],
                                     lhsT=ons[c_][:, ts_], rhs=wos_sb[:, hs],
                                     start=False, stop=True)
                if tail:
                    # ACT is exp-free here: split the copies across engines
                    # and flush each q-tile as soon as it is staged
                    nc.scalar.activation(out=otc[c_][:, tt, 0:384],
                                         in_=Sg[:, 0:384], func=COPY)
                    nc.vector.tensor_copy(out=otc[c_][:, tt, 384:768],
                                          in_=Sg[:, 512:896])
                    nc.sync.dma_start(
                        out=y[128 * t:128 * (t + 1), :].rearrange(
                            "(t p) d -> p t d", p=128),
                        in_=otc[c_][:, tt:tt + 1, :])
                    return
                nc.vector.tensor_copy(out=otc[c_][:, tt, 0:384], in_=Sg[:, 0:384])
                nc.vector.tensor_copy(out=otc[c_][:, tt, 384:768],
                                      in_=Sg[:, 512:896])
                if tt % 2 == 1:  # flush 2 q-tiles
                    nc.sync.dma_start(
                        out=y[128 * (t - 1):128 * (t + 1), :].rearrange(
                            "(t p) d -> p t d", p=128),
                        in_=otc[c_][:, tt - 1:tt + 1, :])

            def att(c, fillers):
                npairs = 2 * c + 2
                pts = {}
                po_t = {}

                def emit_S(h, p):
                    Sg = big.tile([128, 1024], F32, tag="big", name=f"S{c}{h}{p}")
                    pt = ptp.tile([128, 1024], BF16, tag="pt", name=f"pt{c}{h}{p}")
                    if p < 2 * c:          # full pair: j = 2p, 2p+1
                        j0 = 2 * p
                        nc.tensor.matmul(
                            out=Sg[:, 0:512], lhsT=kq(h, j0),
                            rhs=qv(h, c, 0, 512), start=True, stop=True)
                        nc.tensor.matmul(
                            out=Sg[:, 512:1024], lhsT=kq(h, j0 + 1),
                            rhs=qv(h, c, 0, 512), start=True, stop=True)
                        exp_emit(pt, Sg, [(0, 512, j0), (512, 1024, j0 + 1)])
                    elif p == 2 * c:       # diag A: j=4c (512 cols), 4c+1 (384)
                        j0 = 4 * c
                        nc.tensor.matmul(
                            out=Sg[:, 0:512], lhsT=kq(h, j0),
                            rhs=qv(h, c, 0, 512), start=True, stop=True)
                        nc.tensor.matmul(
                            out=Sg[:, 512:896], lhsT=kq(h, j0 + 1),
                            rhs=qv(h, c, 128, 512), start=True, stop=True)
                        exp_emit(pt, Sg, [(0, 512, j0), (512, 896, j0 + 1)])
                        tri_mask(pt, 512)
                    else:                  # diag B: j=4c+2 (256 cols), 4c+3
                        # (128, packed at 256:384 -> one bank, one exp)
                        j0 = 4 * c + 2
                        nc.tensor.matmul(
                            out=Sg[:, 0:256], lhsT=kq(h, j0),
                            rhs=qv(h, c, 256, 512), start=True, stop=True)
                        nc.tensor.matmul(
                            out=Sg[:, 256:384], lhsT=kq(h, j0 + 1),
                            rhs=qv(h, c, 384, 512), start=True, stop=True)
                        exp_emit(pt, Sg, [(0, 256, j0), (256, 384, j0 + 1)])
                        tri_mask(pt, 256)
                    pts[(h, p)] = pt

                def emit_P_main(h, p):
                    if c == 0:
                        return  # all PVs deferred (need the masked pt anyway)
                    pt = pts[(h, p)]
                    if p == 0:
                        po_t[h] = pop.tile([65, 512], F32, tag="po",
                                           name=f"po{c}_{h}")
                    po = po_t[h]
                    if p < 2 * c:
                        nc.tensor.matmul(
                            out=po[:, :], lhsT=v_ap(h, 2 * p),
                            rhs=pt[:, 0:512], start=(p == 0), stop=False)
                        nc.tensor.matmul(
                            out=po[:, :], lhsT=v_ap(h, 2 * p + 1),
                            rhs=pt[:, 512:1024], start=False, stop=False)
                        pts.pop((h, p))
                    elif p == 2 * c:
                        nc.tensor.matmul(
                            out=po[:, 128:512], lhsT=v_ap(h, 4 * c),
                            rhs=pt[:, 128:512], start=False, stop=False)
                        nc.tensor.matmul(
                            out=po[:, 256:512], lhsT=v_ap(h, 4 * c + 1),
                            rhs=pt[:, 640:896], start=False, stop=False)
                    else:
                        nc.tensor.matmul(
                            out=po[:, 384:512], lhsT=v_ap(h, 4 * c + 2),
                            rhs=pt[:, 128:256], start=False, stop=False)

                def emit_tail(h):
                    """Deferred: triangle PVs (+ all PVs at c==0) + normalize."""
                    ptA = pts.pop((h, 2 * c))
                    ptB = pts.pop((h, 2 * c + 1))
                    if c == 0:
                        po_t[h] = pop.tile([65, 512], F32, tag="po",
                                           name=f"po{c}_{h}")
                        po = po_t[h]
                        nc.tensor.matmul(
                            out=po[:, 0:512], lhsT=v_ap(h, 0),
                            rhs=ptA[:, 0:512], start=True, stop=False)
                        nc.tensor.matmul(
                            out=po[:, 128:512], lhsT=v_ap(h, 1),
                            rhs=ptA[:, 512:896], start=False, stop=False)
                        nc.tensor.matmul(
                            out=po[:, 256:512], lhsT=v_ap(h, 2),
                            rhs=ptB[:, 0:256], start=False, stop=False)
                        nc.tensor.matmul(
                            out=po[:, 384:512], lhsT=v_ap(h, 3),
                            rhs=ptB[:, 256:384], start=False, stop=True)
                    else:
                        po = po_t[h]
                        nc.tensor.matmul(
                            out=po[:, 0:128], lhsT=v_ap(h, 4 * c),
                            rhs=ptA[:, 0:128], start=False, stop=False)
                        nc.tensor.matmul(
                            out=po[:, 128:256], lhsT=v_ap(h, 4 * c + 1),
                            rhs=ptA[:, 512:640], start=False, stop=False)
                        nc.tensor.matmul(
                            out=po[:, 256:384], lhsT=v_ap(h, 4 * c + 2),
                            rhs=ptB[:, 0:128], start=False, stop=False)
                        nc.tensor.matmul(
                            out=po[:, 384:512], lhsT=v_ap(h, 4 * c + 3),
                            rhs=ptB[:, 256:384], start=False, stop=True)
                    # stage po to SBUF at once (frees the PSUM bank fast);
                    # the slow recip/bcast/mul chain then runs off-path
                    posb = posp.tile([65, 512], F32, tag="pos", name=f"ps{c}{h}")
                    if c == 3:  # ACT is exp-free by the chunk-3 tails
                        nc.scalar.activation(out=posb[:], in_=po[:, :], func=COPY)
                    else:
                        nc.vector.tensor_copy(out=posb[:], in_=po[:, :])
                    den = recp.tile([1, 512], F32, tag="den", name=f"dn{c}{h}")
                    nc.vector.tensor_copy(out=den[:], in_=po[64:65, :])
                    rec = recp.tile([1, 512], F32, tag="rec", name=f"rc{c}{h}")
                    nc.vector.reciprocal_approx_fast(out=rec[:], in_=den[:])
                    recb = recp.tile([1, 512], BF16, tag="recb", name=f"rb{c}{h}")
                    if c == 3:
                        nc.scalar.activation(out=recb[:], in_=rec[:], func=COPY)
                    else:
                        nc.vector.tensor_copy(out=recb[:], in_=rec[:])
                    # broadcast via PE rank-1 outer product: ones65^T @ recb
                    # (keeps the GpSimd FIFO free for the causal-mask multiplies)
                    bc = pop.tile([65, 512], F32, tag="po", name=f"bc{c}{h}")
                    nc.tensor.matmul(out=bc[:, :], lhsT=ones65[:],
                                     rhs=recb[:], start=True, stop=True)
                    if h == 0:
                        nc.vector.tensor_mul(out=onp[c][0:64, :],
                                             in0=posb[0:64, :], in1=bc[0:64, :])
                    elif h == 1:
                        nc.vector.tensor_mul(out=onp[c][64:128, :],
                                             in0=posb[0:64, :], in1=bc[0:64, :])
                    else:
                        nc.vector.tensor_mul(out=ons[c][:],
                                             in0=posb[:, :], in1=bc[:, :])

                units = [(h, p) for h in range(HPC) for p in range(npairs)]
                nu = len(units)
                nf = len(fillers)
                fill_at = {}
                for k in range(nf):
                    fill_at.setdefault(
                        min(nu - 1, (k + 1) * nu // (nf + 1)), []).append(k)
                pend = deque()
                tails = deque()

                def pop_one():
                    h, p = pend.popleft()
                    emit_P_main(h, p)
                    if p == npairs - 1:
                        tails.append(h)
                    elif p == 1 and tails:
                        emit_tail(tails.popleft())

                for i, u in enumerate(units):
                    emit_S(*u)
                    pend.append(u)
                    if len(pend) > LAG:
                        pop_one()
                    for k in fill_at.get(i, ()):
                        fillers[k]()
                while pend:
                    pop_one()
                while tails:
                    emit_tail(tails.popleft())

            proj(0)
            nc.sync.dma_start(out=xc[1][:], in_=xT[1][:, :, :])
            nc.scalar.dma_start(out=xc[2][:], in_=xT[2][:, :, :])
            proj(1)
            nc.sync.dma_start(out=xc[3][:], in_=xT[3][:, :, :])
            nc.scalar.dma_start(out=tri_sb[:], in_=trid[:, :, :])
            nc.sync.dma_start(out=wop_sb[:], in_=wop[:, :])
            nc.scalar.dma_start(out=wos_sb[:], in_=wos[:, :])
            if use_pbias:
                nc.sync.dma_start(out=pb_sb[:], in_=pbias[:, :])
            att(0, proj_groups(2, on_act=False))
            att(1, proj_groups(3, on_act=False))
            att(2, [lambda t=t: outproj(t) for t in (0, 1, 2, 3)])
            att(3, [lambda t=t: outproj(t) for t in (4, 5, 6, 7, 8, 9, 10, 11)])
            for t in (12, 13, 14, 15):
                outproj(t, tail=True)

    nc.compile()
    return nc


def make_inputs(x, attention_mask, Wq, Wk, Wv, Wo, bo, use_pbias):
    bf = ml_dtypes.bfloat16
    kk = np.arange(128)[:, None]
    qq = np.arange(128)[None, :]
    tri01 = (qq >= kk).astype(np.float32)
    tri2 = np.repeat(tri01[:, None, :], 2, axis=1)

    def split_k(arr):  # [768, C] -> [128, 6, C]
        return np.ascontiguousarray(
            arr.reshape(KT, 128, arr.shape[1]).transpose(1, 0, 2))

    in_maps = []
    for core in range(NCORES):
        b, g = core // 4, core % 4
        h0, h1, h2 = range(HPC * g, HPC * (g + 1))
        xTb = split_k(np.ascontiguousarray(x[b].T)).astype(bf)
        wqk = np.empty((D, 384), np.float32)
        wqk[:, 0:64] = Wq[HD * h0:HD * (h0 + 1), :].T
        wqk[:, 64:128] = Wq[HD * h1:HD * (h1 + 1), :].T
        wqk[:, 128:192] = Wk[HD * h0:HD * (h0 + 1), :].T
        wqk[:, 192:256] = Wk[HD * h1:HD * (h1 + 1), :].T
        wqk[:, 256:320] = Wq[HD * h2:HD * (h2 + 1), :].T
        wqk[:, 320:384] = Wk[HD * h2:HD * (h2 + 1), :].T
        wv_ = Wv[HD * h0:HD * (h2 + 1), :].T
        wop = np.concatenate(
            [Wo[:, HD * h0:HD * (h0 + 1)].T, Wo[:, HD * h1:HD * (h1 + 1)].T])
        wos = np.zeros((65, D), np.float32)
        wos[0:64] = Wo[:, HD * h2:HD * (h2 + 1)].T
        if g == 0:
            wos[64] = bo
        m = {"wqk": split_k(wqk).astype(bf),
             "wv": split_k(np.ascontiguousarray(wv_)).astype(bf),
             "wop": wop.astype(bf),
             "wos": wos.astype(bf),
             "trid": tri2.astype(bf)}
        for c in range(CH):
            m[f"xT{c}"] = np.ascontiguousarray(xTb[:, :, 512 * c:512 * (c + 1)])
        if use_pbias:
            pb = ((1.0 - attention_mask[b].astype(np.float32)) * NEG)
            m["pbias"] = np.ascontiguousarray(pb.reshape(SQT, 128).T)
        in_maps.append(m)
    return in_maps


_NC_CACHE = {}


def _get_nc(use_pbias):
    key = ("nc", use_pbias)
    if key not in _NC_CACHE:
        _NC_CACHE[key] = build_nc(use_pbias)
    return _NC_CACHE[key]


def kernel(x, attention_mask, Wq, Wk, Wv, Wo, bo, _trace=False, _trace_kwargs=None):
    x = np.asarray(x, np.float32)
    attention_mask = np.asarray(attention_mask, np.float32)
    Wq, Wk, Wv, Wo, bo = (np.asarray(a, np.float32) for a in (Wq, Wk, Wv, Wo, bo))
    use_pbias = not bool(np.all(attention_mask == 1.0))
    nc = _get_nc(use_pbias)
    in_maps = make_inputs(x, attention_mask, Wq, Wk, Wv, Wo, bo, use_pbias)
    res = run_bass_kernel_spmd(nc, in_maps, list(range(NCORES)),
                               trace=_trace, **(_trace_kwargs or {}))
    parts = [np.asarray(res.results[i]["y"]).astype(np.float32)
             for i in range(NCORES)]
    out = np.stack([sum(parts[0:4]), sum(parts[4:8])])
    if _trace:
        return out, res
    return out



# revision 45
# speedup vs baseline: 1.0191x; 1.0191x over previous
"""GPT-Neo self-attention on 8 NeuronCores (Trainium2, Bass/Tile) — v10.

Sharding: core i handles batch i//4 and head-group i%4 (3 of 12 heads).
Each core computes a partial out-projection [S, D] (bf16); host sums the
4 partials per batch in f32.

v5 vs v4:
- Per-chunk SBUF tiles for q/k/v/onorm/ot so the tile-granularity
  dependency tracker cannot create false cross-chunk waits (PE matmuls
  were stalling on unrelated copies into shared tiles).
- Triangle PVs + normalize for head h are DEFERRED one head: their
  exp -> GpSimd-mask -> PV chain gets a whole head of slack, so the
  in-order PE stream never waits on the Pool/DVE queues.
- Ascending chunks; proj(c+2) psum-groups and out-projections run as
  PE fillers inside att windows (they need no ACT work, balancing the
  exp-bound attention phases).
"""

import numpy as np
import ml_dtypes
from collections import deque
from contextlib import ExitStack

import concourse.bass as bass
from concourse import bacc
import concourse.mybir as mybir
import concourse.tile as tile
from concourse.bass_utils import run_bass_kernel_spmd

B, S, D, H = 2, 2048, 768, 12
HD = 64
HPC = 3
NCORES = 8
NEG = -1.0e30
F32 = mybir.dt.float32
BF16 = mybir.dt.bfloat16
EXP = mybir.ActivationFunctionType.Exp
COPY = mybir.ActivationFunctionType.Copy

KT = D // 128
SQT = S // 128
CH = S // 512
LAG = 4          # main-PV pair-units behind scores


def build_nc(use_pbias=False):
    nc = bacc.Bacc(None, target_bir_lowering=False)

    xT = [nc.declare_dram_parameter(f"xT{c}", [128, KT, 512], BF16,
                                    isOutput=False) for c in range(CH)]
    wqk = nc.declare_dram_parameter("wqk", [128, KT, 384], BF16, isOutput=False)
    wv = nc.declare_dram_parameter("wv", [128, KT, 192], BF16, isOutput=False)
    wop = nc.declare_dram_parameter("wop", [128, D], BF16, isOutput=False)
    wos = nc.declare_dram_parameter("wos", [65, D], BF16, isOutput=False)
    trid = nc.declare_dram_parameter("trid", [128, 128], BF16, isOutput=False)
    if use_pbias:
        pbias = nc.declare_dram_parameter("pbias", [128, SQT], F32, isOutput=False)
    y = nc.declare_dram_parameter("y", [S, D], BF16, isOutput=True)

    with tile.TileContext(nc) as tc:
        with ExitStack() as ctx:
            persist = ctx.enter_context(tc.tile_pool(name="persist", bufs=1))
            ptp = ctx.enter_context(tc.tile_pool(name="ptp", bufs=12))
            recp = ctx.enter_context(tc.tile_pool(name="recp", bufs=3))
            posp = ctx.enter_context(tc.tile_pool(name="posp", bufs=2))
            bcp = ctx.enter_context(tc.tile_pool(name="bcp", bufs=2))
            big = ctx.enter_context(tc.tile_pool(name="big", bufs=3, space="PSUM"))
            pop = ctx.enter_context(tc.tile_pool(name="pop", bufs=2, space="PSUM"))

            xc = [persist.tile([128, KT, 512], BF16, tag=f"xc{c}", name=f"xc{c}")
                  for c in range(CH)]
            wqk_sb = persist.tile([128, KT, 384], BF16, tag="wqk", name="wqk")
            wv_sb = persist.tile([128, KT, 192], BF16, tag="wv", name="wv")
            wop_sb = persist.tile([128, D], BF16, tag="wop", name="wop")
            wos_sb = persist.tile([65, D], BF16, tag="wos", name="wos")
            tri_sb = persist.tile([128, 128], BF16, tag="tri", name="tri")
            ones65 = persist.tile([1, 65], BF16, tag="ones65", name="ones65")
            if use_pbias:
                pb_sb = persist.tile([128, SQT], F32, tag="pb", name="pb")
            # per-chunk activation tiles (avoid false tile-level deps)
            q01c = [persist.tile([128, 512], BF16, tag=f"q01_{c}", name=f"q01_{c}")
                    for c in range(CH)]
            k01c = [persist.tile([128, 512], BF16, tag=f"k01_{c}", name=f"k01_{c}")
                    for c in range(CH)]
            q2c = [persist.tile([64, 512], BF16, tag=f"q2_{c}", name=f"q2_{c}")
                   for c in range(CH)]
            k2c = [persist.tile([64, 512], BF16, tag=f"k2_{c}", name=f"k2_{c}")
                   for c in range(CH)]
            vc = [persist.tile([128, HPC, 4, 65], BF16, tag=f"v{c}", name=f"v{c}")
                  for c in range(CH)]
            onp = [persist.tile([128, 512], BF16, tag=f"onp{c}", name=f"onp{c}")
                   for c in range(CH)]
            ons = [persist.tile([65, 512], BF16, tag=f"ons{c}", name=f"ons{c}")
                   for c in range(CH)]
            otc = [persist.tile([128, 4, D], BF16, tag=f"ot{c}", name=f"ot{c}")
                   for c in range(CH)]

            # ---- input DMAs on both DGE queues ----
            # only proj(0)'s inputs before the first matmuls: the first
            # compute's DMA-counter wait then covers just these transfers
            nc.sync.dma_start(out=wqk_sb[:], in_=wqk[:, :, :])
            nc.scalar.dma_start(out=wv_sb[:], in_=wv[:, :, :])
            nc.sync.dma_start(out=xc[0][:, 0:3, :], in_=xT[0][:, 0:3, :])
            nc.scalar.dma_start(out=xc[0][:, 3:6, :], in_=xT[0][:, 3:6, :])
            for c in range(CH):
                nc.vector.memset(vc[c][:], 1.0)
            nc.vector.memset(ones65[:], 1.0)

            def proj_qk_group(c, off, on_act=True):
                ps = big.tile([128, 1024], F32, tag="big", name=f"p{off}_{c}")
                for k in range(KT):
                    nc.tensor.matmul(
                        out=ps[:, 0:512],
                        lhsT=wqk_sb[:, k, off:off + 128],
                        rhs=xc[c][:, k, :],
                        start=(k == 0), stop=(k == KT - 1))
                cp = (nc.scalar.copy if on_act else
                      (lambda out, in_: nc.vector.tensor_copy(out=out, in_=in_)))
                if off == 0:
                    cp(out=q01c[c][:], in_=ps[:, 0:512])
                elif off == 128:
                    cp(out=k01c[c][:], in_=ps[:, 0:512])
                else:
                    cp(out=q2c[c][:], in_=ps[0:64, 0:512])
                    nc.vector.tensor_copy(out=k2c[c][:], in_=ps[64:128, 0:512])

            def proj_v_group(c, jj):
                pv = big.tile([128, 1024], F32, tag="big", name=f"pv{c}_{jj}")
                for k in range(KT):
                    nc.tensor.matmul(
                        out=pv[:, 0:192],
                        lhsT=xc[c][:, k, 128 * jj:128 * (jj + 1)],
                        rhs=wv_sb[:, k, :],
                        start=(k == 0), stop=(k == KT - 1))
                for h in range(HPC):
                    nc.vector.tensor_copy(
                        out=vc[c][:, h, jj, 0:64],
                        in_=pv[:, 64 * h:64 * (h + 1)])

            def proj_groups(c, on_act=True):
                gs = [lambda off=off: proj_qk_group(c, off, on_act)
                      for off in (0, 128, 256)]
                gs += [lambda jj=jj: proj_v_group(c, jj) for jj in range(4)]
                return gs

            def proj(c):
                for g in proj_groups(c):
                    g()

            def kq(h, j):
                """(k-block lhsT, q-chunk-view fn) for head h, key tile j."""
                cj, jj = j // 4, j % 4
                if h == 2:
                    return k2c[cj][:, 128 * jj:128 * (jj + 1)]
                lo = 64 * h
                return k01c[cj][lo:lo + 64, 128 * jj:128 * (jj + 1)]

            def qv(h, c, lo, hi):
                if h == 2:
                    return q2c[c][:, lo:hi]
                p0 = 64 * h
                return q01c[c][p0:p0 + 64, lo:hi]

            def v_ap(h, j):
                return vc[j // 4][:, h, j % 4, :]

            def exp_emit(pt, Sg, sections):
                if use_pbias:
                    for lo, hi, j in sections:
                        nc.scalar.activation(out=pt[:, lo:hi], in_=Sg[:, lo:hi],
                                             func=EXP, bias=pb_sb[:, j:j + 1])
                else:
                    lo, hi = sections[0][0], sections[-1][1]
                    nc.scalar.activation(out=pt[:, lo:hi], in_=Sg[:, lo:hi],
                                         func=EXP)

            def outproj(t, tail=False):
                c_, tt = t // 4, t % 4
                Sg = big.tile([128, 1024], F32, tag="big", name=f"op{t}")
                ts_ = slice(128 * tt, 128 * (tt + 1))
                for lo in (0, 512):
                    hs = slice(384 * (lo // 512), 384 * (lo // 512) + 384)
                    nc.tensor.matmul(out=Sg[:, lo:lo + 384],
                                     lhsT=onp[c_][:, ts_], rhs=wop_sb[:, hs],
                                     start=True, stop=False)
                    nc.tensor.matmul(out=Sg[:, lo:lo + 384],
                                     lhsT=ons[c_][:, ts_], rhs=wos_sb[:, hs],
                                     start=False, stop=True)
                if tail:
                    # ACT is exp-free here: split the copies across engines
                    # and flush each q-tile as soon as it is staged
                    nc.scalar.activation(out=otc[c_][:, tt, 0:384],
                                         in_=Sg[:, 0:384], func=COPY)
                    nc.vector.tensor_copy(out=otc[c_][:, tt, 384:768],
                                          in_=Sg[:, 512:896])
                    nc.sync.dma_start(
                        out=y[128 * t:128 * (t + 1), :].rearrange(
                            "(t p) d -> p t d", p=128),
                        in_=otc[c_][:, tt:tt + 1, :])
                    return
                nc.vector.tensor_copy(out=otc[c_][:, tt, 0:384], in_=Sg[:, 0:384])
                nc.vector.tensor_copy(out=otc[c_][:, tt, 384:768],
                                      in_=Sg[:, 512:896])
                if tt % 2 == 1:  # flush 2 q-tiles
                    nc.sync.dma_start(
                        out=y[128 * (t - 1):128 * (t + 1), :].rearrange(
                            "(t p) d -> p t d", p=128),
                        in_=otc[c_][:, tt - 1:tt + 1, :])

            def att(c, fillers):
                npairs = 2 * c + 2
                pts = {}
                po_t = {}

                def emit_S(h, p):
                    Sg = big.tile([128, 1024], F32, tag="big", name=f"S{c}{h}{p}")
                    pt = ptp.tile([128, 1024], BF16, tag="pt", name=f"pt{c}{h}{p}")
                    if p < 2 * c:          # full pair: j = 2p, 2p+1
                        j0 = 2 * p
                        nc.tensor.matmul(
                            out=Sg[:, 0:512], lhsT=kq(h, j0),
                            rhs=qv(h, c, 0, 512), start=True, stop=True)
                        nc.tensor.matmul(
                            out=Sg[:, 512:1024], lhsT=kq(h, j0 + 1),
                            rhs=qv(h, c, 0, 512), start=True, stop=True)
                        exp_emit(pt, Sg, [(0, 512, j0), (512, 1024, j0 + 1)])
                    elif p == 2 * c:       # diag A: j=4c (512 cols), 4c+1 (384)
                        j0 = 4 * c
                        nc.tensor.matmul(
                            out=Sg[:, 0:512], lhsT=kq(h, j0),
                            rhs=qv(h, c, 0, 512), start=True, stop=True)
                        nc.tensor.matmul(
                            out=Sg[:, 512:896], lhsT=kq(h, j0 + 1),
                            rhs=qv(h, c, 128, 512), start=True, stop=True)
                        exp_emit(pt, Sg, [(0, 512, j0), (512, 896, j0 + 1)])
                        nc.gpsimd.tensor_mul(out=pt[:, 0:128], in0=pt[:, 0:128],
                                             in1=tri_sb[:])
                        nc.gpsimd.tensor_mul(out=pt[:, 512:640],
                                             in0=pt[:, 512:640], in1=tri_sb[:])
                    else:                  # diag B: j=4c+2 (256 cols), 4c+3 (128)
                        j0 = 4 * c + 2
                        nc.tensor.matmul(
                            out=Sg[:, 0:256], lhsT=kq(h, j0),
                            rhs=qv(h, c, 256, 512), start=True, stop=True)
                        nc.tensor.matmul(
                            out=Sg[:, 512:640], lhsT=kq(h, j0 + 1),
                            rhs=qv(h, c, 384, 512), start=True, stop=True)
                        if use_pbias:
                            exp_emit(pt, Sg, [(0, 256, j0), (512, 640, j0 + 1)])
                        else:
                            nc.scalar.activation(out=pt[:, 0:256],
                                                 in_=Sg[:, 0:256], func=EXP)
                            nc.scalar.activation(out=pt[:, 512:640],
                                                 in_=Sg[:, 512:640], func=EXP)
                        nc.gpsimd.tensor_mul(out=pt[:, 0:128], in0=pt[:, 0:128],
                                             in1=tri_sb[:])
                        nc.gpsimd.tensor_mul(out=pt[:, 512:640],
                                             in0=pt[:, 512:640], in1=tri_sb[:])
                    pts[(h, p)] = pt

                def emit_P_main(h, p):
                    if c == 0:
                        return  # all PVs deferred (need the masked pt anyway)
                    pt = pts[(h, p)]
                    if p == 0:
                        po_t[h] = pop.tile([65, 512], F32, tag="po",
                                           name=f"po{c}_{h}")
                    po = po_t[h]
                    if p < 2 * c:
                        nc.tensor.matmul(
                            out=po[:, :], lhsT=v_ap(h, 2 * p),
                            rhs=pt[:, 0:512], start=(p == 0), stop=False)
                        nc.tensor.matmul(
                            out=po[:, :], lhsT=v_ap(h, 2 * p + 1),
                            rhs=pt[:, 512:1024], start=False, stop=False)
                        pts.pop((h, p))
                    elif p == 2 * c:
                        nc.tensor.matmul(
                            out=po[:, 128:512], lhsT=v_ap(h, 4 * c),
                            rhs=pt[:, 128:512], start=False, stop=False)
                        nc.tensor.matmul(
                            out=po[:, 256:512], lhsT=v_ap(h, 4 * c + 1),
                            rhs=pt[:, 640:896], start=False, stop=False)
                    else:
                        nc.tensor.matmul(
                            out=po[:, 384:512], lhsT=v_ap(h, 4 * c + 2),
                            rhs=pt[:, 128:256], start=False, stop=False)

                def emit_tail(h):
                    """Deferred: triangle PVs (+ all PVs at c==0) + normalize."""
                    ptA = pts.pop((h, 2 * c))
                    ptB = pts.pop((h, 2 * c + 1))
                    if c == 0:
                        po_t[h] = pop.tile([65, 512], F32, tag="po",
                                           name=f"po{c}_{h}")
                        po = po_t[h]
                        nc.tensor.matmul(
                            out=po[:, 0:512], lhsT=v_ap(h, 0),
                            rhs=ptA[:, 0:512], start=True, stop=False)
                        nc.tensor.matmul(
                            out=po[:, 128:512], lhsT=v_ap(h, 1),
                            rhs=ptA[:, 512:896], start=False, stop=False)
                        nc.tensor.matmul(
                            out=po[:, 256:512], lhsT=v_ap(h, 2),
                            rhs=ptB[:, 0:256], start=False, stop=False)
                        nc.tensor.matmul(
                            out=po[:, 384:512], lhsT=v_ap(h, 3),
                            rhs=ptB[:, 512:640], start=False, stop=True)
                    else:
                        po = po_t[h]
                        nc.tensor.matmul(
                            out=po[:, 0:128], lhsT=v_ap(h, 4 * c),
                            rhs=ptA[:, 0:128], start=False, stop=False)
                        nc.tensor.matmul(
                            out=po[:, 128:256], lhsT=v_ap(h, 4 * c + 1),
                            rhs=ptA[:, 512:640], start=False, stop=False)
                        nc.tensor.matmul(
                            out=po[:, 256:384], lhsT=v_ap(h, 4 * c + 2),
                            rhs=ptB[:, 0:128], start=False, stop=False)
                        nc.tensor.matmul(
                            out=po[:, 384:512], lhsT=v_ap(h, 4 * c + 3),
                            rhs=ptB[:, 512:640], start=False, stop=True)
                    # stage po to SBUF at once (frees the PSUM bank fast);
                    # the slow recip/bcast/mul chain then runs off-path
                    posb = posp.tile([65, 512], F32, tag="pos", name=f"ps{c}{h}")
                    if c == 3:  # ACT is exp-free by the chunk-3 tails
                        nc.scalar.activation(out=posb[:], in_=po[:, :], func=COPY)
                    else:
                        nc.vector.tensor_copy(out=posb[:], in_=po[:, :])
                    den = recp.tile([1, 512], F32, tag="den", name=f"dn{c}{h}")
                    nc.vector.tensor_copy(out=den[:], in_=po[64:65, :])
                    rec = recp.tile([1, 512], F32, tag="rec", name=f"rc{c}{h}")
                    nc.vector.reciprocal_approx_fast(out=rec[:], in_=den[:])
                    recb = recp.tile([1, 512], BF16, tag="recb", name=f"rb{c}{h}")
                    if c == 3:
                        nc.scalar.activation(out=recb[:], in_=rec[:], func=COPY)
                    else:
                        nc.vector.tensor_copy(out=recb[:], in_=rec[:])
                    # broadcast via PE rank-1 outer product: ones65^T @ recb
                    # (keeps the GpSimd FIFO free for the causal-mask multiplies)
                    bc = pop.tile([65, 512], F32, tag="po", name=f"bc{c}{h}")
                    nc.tensor.matmul(out=bc[:, :], lhsT=ones65[:],
                                     rhs=recb[:], start=True, stop=True)
                    if h == 0:
                        nc.vector.tensor_mul(out=onp[c][0:64, :],
                                             in0=posb[0:64, :], in1=bc[0:64, :])
                    elif h == 1:
                        nc.vector.tensor_mul(out=onp[c][64:128, :],
                                             in0=posb[0:64, :], in1=bc[0:64, :])
                    else:
                        nc.vector.tensor_mul(out=ons[c][:],
                                             in0=posb[:, :], in1=bc[:, :])

                units = [(h, p) for h in range(HPC) for p in range(npairs)]
                nu = len(units)
                nf = len(fillers)
                fill_at = {}
                for k in range(nf):
                    fill_at.setdefault(
                        min(nu - 1, (k + 1) * nu // (nf + 1)), []).append(k)
                pend = deque()
                tails = deque()

                def pop_one():
                    h, p = pend.popleft()
                    emit_P_main(h, p)
                    if p == npairs - 1:
                        tails.append(h)
                    elif p == 1 and tails:
                        emit_tail(tails.popleft())

                for i, u in enumerate(units):
                    emit_S(*u)
                    pend.append(u)
                    if len(pend) > LAG:
                        pop_one()
                    for k in fill_at.get(i, ()):
                        fillers[k]()
                while pend:
                    pop_one()
                while tails:
                    emit_tail(tails.popleft())

            proj(0)
            nc.sync.dma_start(out=xc[1][:], in_=xT[1][:, :, :])
            nc.scalar.dma_start(out=xc[2][:], in_=xT[2][:, :, :])
            proj(1)
            nc.sync.dma_start(out=xc[3][:], in_=xT[3][:, :, :])
            nc.scalar.dma_start(out=tri_sb[:], in_=trid[:, :])
            nc.sync.dma_start(out=wop_sb[:], in_=wop[:, :])
            nc.scalar.dma_start(out=wos_sb[:], in_=wos[:, :])
            if use_pbias:
                nc.sync.dma_start(out=pb_sb[:], in_=pbias[:, :])
            att(0, proj_groups(2, on_act=False))
            att(1, proj_groups(3, on_act=False))
            att(2, [lambda t=t: outproj(t) for t in (0, 1, 2, 3)])
            att(3, [lambda t=t: outproj(t) for t in (4, 5, 6, 7, 8, 9, 10, 11)])
            for t in (12, 13, 14, 15):
                outproj(t, tail=True)

    nc.compile()
    return nc


def make_inputs(x, attention_mask, Wq, Wk, Wv, Wo, bo, use_pbias):
    bf = ml_dtypes.bfloat16
    kk = np.arange(128)[:, None]
    qq = np.arange(128)[None, :]
    tri01 = (qq >= kk).astype(np.float32)

    def split_k(arr):  # [768, C] -> [128, 6, C]
        return np.ascontiguousarray(
            arr.reshape(KT, 128, arr.shape[1]).transpose(1, 0, 2))

    in_maps = []
    for core in range(NCORES):
        b, g = core // 4, core % 4
        h0, h1, h2 = range(HPC * g, HPC * (g + 1))
        xTb = split_k(np.ascontiguousarray(x[b].T)).astype(bf)
        wqk = np.empty((D, 384), np.float32)
        wqk[:, 0:64] = Wq[HD * h0:HD * (h0 + 1), :].T
        wqk[:, 64:128] = Wq[HD * h1:HD * (h1 + 1), :].T
        wqk[:, 128:192] = Wk[HD * h0:HD * (h0 + 1), :].T
        wqk[:, 192:256] = Wk[HD * h1:HD * (h1 + 1), :].T
        wqk[:, 256:320] = Wq[HD * h2:HD * (h2 + 1), :].T
        wqk[:, 320:384] = Wk[HD * h2:HD * (h2 + 1), :].T
        wv_ = Wv[HD * h0:HD * (h2 + 1), :].T
        wop = np.concatenate(
            [Wo[:, HD * h0:HD * (h0 + 1)].T, Wo[:, HD * h1:HD * (h1 + 1)].T])
        wos = np.zeros((65, D), np.float32)
        wos[0:64] = Wo[:, HD * h2:HD * (h2 + 1)].T
        if g == 0:
            wos[64] = bo
        m = {"wqk": split_k(wqk).astype(bf),
             "wv": split_k(np.ascontiguousarray(wv_)).astype(bf),
             "wop": wop.astype(bf),
             "wos": wos.astype(bf),
             "trid": tri01.astype(bf)}
        for c in range(CH):
            m[f"xT{c}"] = np.ascontiguousarray(xTb[:, :, 512 * c:512 * (c + 1)])
        if use_pbias:
            pb = ((1.0 - attention_mask[b].astype(np.float32)) * NEG)
            m["pbias"] = np.ascontiguousarray(pb.reshape(SQT, 128).T)
        in_maps.append(m)
    return in_maps


_NC_CACHE = {}


def _get_nc(use_pbias):
    key = ("nc", use_pbias)
    if key not in _NC_CACHE:
        _NC_CACHE[key] = build_nc(use_pbias)
    return _NC_CACHE[key]


def kernel(x, attention_mask, Wq, Wk, Wv, Wo, bo, _trace=False, _trace_kwargs=None):
    x = np.asarray(x, np.float32)
    attention_mask = np.asarray(attention_mask, np.float32)
    Wq, Wk, Wv, Wo, bo = (np.asarray(a, np.float32) for a in (Wq, Wk, Wv, Wo, bo))
    use_pbias = not bool(np.all(attention_mask == 1.0))
    nc = _get_nc(use_pbias)
    in_maps = make_inputs(x, attention_mask, Wq, Wk, Wv, Wo, bo, use_pbias)
    res = run_bass_kernel_spmd(nc, in_maps, list(range(NCORES)),
                               trace=_trace, **(_trace_kwargs or {}))
    parts = [np.asarray(res.results[i]["y"]).astype(np.float32)
             for i in range(NCORES)]
    out = np.stack([sum(parts[0:4]), sum(parts[4:8])])
    if _trace:
        return out, res
    return out

